# revision 1
# baseline (speedup 1.0000x reference)
"""v15: direct HBM->HBM rolled-window copies (no SBUF staging).

Problem: x [64, 3, 512, 512] f32, shifts [64, 2] int32 in [-16, 16].
out[b, c, h, w] = x[b, c, (h - shifts[b,0]) % 512, (w - shifts[b,1]) % 512]

Host-side, each channel is circular-padded by 16 px (544x544, flat rows
of a [24, 296000] tensor). Both rolls collapse into a window read at
element offset (16-sh)*544 + (16-sw). v14 staged windows through SBUF
(load + store), pushing 2x the bytes through the SDMA engines and
saturating the 436 GB/s SBUF-fabric ceiling at 96%. Here each (batch,
channel) is ONE DRAM->DRAM DMA: out[b,c] (contiguous [512,512]) <-
padded window ([512 rows @ 544 stride, 512 cols], dynamic offset).
Equal dim-0 counts (512 rows both sides) hit the DGE's HbmToHbm
reshape: descriptors fan across all 16 SDMA engines, and each byte
crosses an engine once instead of twice.

No SBUF tiles, no slot semaphores -- batches are fully independent;
each engine just drains its 12 copy DMAs and waits for its completion
count. Dynamic-offset DMAs leak sequencer registers at trace time
(49/engine budget), so the 24 copies split across the two HWDGE
sequencers, offsets computed with in-place reg ALU on reused registers
(constants hoisted; see memory notes).
"""

from contextlib import ExitStack

import numpy as np

import concourse.bass as bass
import concourse.mybir as mybir
from bass_rust import RegisterHandles, make_scalar_value
from concourse.bass_utils import run_bass_kernel_spmd

B_TOTAL, C, H, W = 64, 3, 512, 512
N_CORES = 8
B = B_TOTAL // N_CORES
MAX_SHIFT = 16
PAD = 2 * MAX_SHIFT  # 32
HP, WP = H + PAD, W + PAD  # 544, 544
CH_ELEMS = HP * WP  # 295936
CH_STRIDE = CH_ELEMS + 64  # 296000, tail pad keeps max window in bounds
WIN = H * WP  # 278528: window covering 512 padded rows
MAX_OFF = PAD * WP + PAD  # 17440


def build_kernel():
    nc = bass.Bass()
    x = nc.dram_tensor("x", [B * C, CH_STRIDE], mybir.dt.float32, kind="ExternalInput")
    shifts = nc.dram_tensor("shifts", [B, 2], mybir.dt.int32, kind="ExternalInput")
    out = nc.dram_tensor("out", [B, C, H, W], mybir.dt.float32, kind="ExternalOutput")

    with (
        nc.sbuf_tensor([1, 3 * B * 2], mybir.dt.int32) as sb_shifts,
        nc.semaphore("pre_sem_sp") as pre_sem_sp,
        nc.semaphore("pre_sem_act") as pre_sem_act,
        nc.semaphore("pre_sem_gp") as pre_sem_gp,
        nc.semaphore("done_sp") as done_sp,
        nc.semaphore("done_act") as done_act,
        nc.semaphore("done_gp") as done_gp,
        ExitStack() as stack,
    ):
        block = stack.enter_context(nc.Block())

        def emit_half(eng, my_batches, sh_base, pre_sem, done_sem):
            eng.dma_start(
                sb_shifts[0:1, sh_base : sh_base + 2 * B],
                shifts.rearrange("b s -> (b s)")[None, :],
            ).then_inc(pre_sem, 16)
            eng.wait_ge(pre_sem, 16)
            n = 0
            with (
                eng.register("r_off") as r_off,
                eng.register("r_sw") as r_sw,
                eng.register("r_cB") as r_cB,
            ):
                # r_off = (16-sh)*544 + (16-sw) = 8720 - (sh*544 + sw)
                eng.reg_mov(r_cB, MAX_SHIFT * WP + MAX_SHIFT)
                for b in my_batches:
                    eng.reg_load(
                        [r_off, r_sw],
                        sb_shifts[0:1, sh_base + 2 * b : sh_base + 2 * b + 2],
                    )
                    eng.reg_mul(r_off, r_off, WP)
                    eng.reg_add(r_off, r_off, r_sw)
                    eng.reg_sub(r_off, r_cB, r_off)
                    rb = make_scalar_value(
                        RegisterHandles([r_off]), min_val=0, max_val=MAX_OFF
                    )
                    for c in range(C):
                        win = x[b * C + c, bass.ds(rb, WIN)]
                        src = win.rearrange("(r w) -> r w", w=WP)[:, 0:W]
                        eng.dma_start(out[b, c], src).then_inc(done_sem, 16)
                        n += 1
            eng.wait_ge(done_sem, 16 * n)

        # three issue queues (qSync, qScalar, qGpSimd): SDMA engines switch
        # queue contexts at packet boundaries, so packets from a third queue
        # can overlap another queue's per-packet gap
        @block.sync
        def _(sync):
            emit_half(sync, [0, 1, 2], 0, pre_sem_sp, done_sp)

        @block.scalar
        def _(scalar):
            emit_half(scalar, [3, 4, 5], 2 * B, pre_sem_act, done_act)

        @block.gpsimd
        def _(gp):
            emit_half(gp, [6, 7], 4 * B, pre_sem_gp, done_gp)

    return nc


_NC_CACHE = None


def _get_nc():
    global _NC_CACHE
    if _NC_CACHE is None:
        _NC_CACHE = build_kernel()
    return _NC_CACHE


def _pad_input(x: np.ndarray) -> np.ndarray:
    """[64, 3, 512, 512] -> [64*3, 296000]: per-channel circular 16-px
    border (544x544) flattened, with 64 tail-pad elems per channel."""
    xp = np.pad(
        x,
        ((0, 0), (0, 0), (MAX_SHIFT, MAX_SHIFT), (MAX_SHIFT, MAX_SHIFT)),
        mode="wrap",
    ).reshape(B_TOTAL * C, CH_ELEMS)
    outp = np.zeros((B_TOTAL * C, CH_STRIDE), dtype=np.float32)
    outp[:, :CH_ELEMS] = xp
    return outp


def kernel(x: np.ndarray, shifts: np.ndarray) -> np.ndarray:
    assert x.shape == (B_TOTAL, C, H, W), x.shape
    assert shifts.shape == (B_TOTAL, 2), shifts.shape
    x = np.ascontiguousarray(x, dtype=np.float32)
    shifts = np.ascontiguousarray(shifts, dtype=np.int32)
    x_pad = _pad_input(x)

    in_maps = [
        {
            "x": x_pad[i * B * C : (i + 1) * B * C],
            "shifts": shifts[i * B : (i + 1) * B],
        }
        for i in range(N_CORES)
    ]
    res = run_bass_kernel_spmd(_get_nc(), in_maps, list(range(N_CORES)))
    return np.concatenate(
        [res.results[i]["out"] for i in range(N_CORES)], axis=0
    ).astype(np.float32)

